# revision 30
# baseline (speedup 1.0000x reference)
"""Fused pre-norm decoder layer (RMSNorm + GQA causal attention w/ RoPE +
RMSNorm + SwiGLU MLP) on 8 Trainium2 NeuronCores.

Sharding: sequence-parallel with folded stripe pairs — core c owns row stripes
{c, 15-c} (128 rows each) so causal attention work is balanced; the MLP is
tensor-parallel (w1/w3 column-split, w2 row-split). Cross-core comms: one
fused AllGather of roped K^T + V (bf16), AllGather of the transposed normed
hidden states (bf16), and a 5-way chunked ReduceScatter of the MLP partial
outputs (bf16, small tail chunk) overlapped with the w2 matmuls.

Attention-side rmsnorm is folded into the projections: the host ships x^T
(bf16) as the stationary operand, the per-row 1/rms scale is folded into the
rope tables (q, k) and the V psum copy, so the QKV matmuls start as soon as
the weights land (no norm / transpose on the critical path).

Self-contained: hardcodes the reference shapes
(B=1, N=2048, DIM=2048, HQ=16, HK=4, HD=128, F=8192).
"""
import numpy as np
import ml_dtypes

import concourse.bass as bass
import concourse.mybir as mybir
import concourse.tile as tile
from concourse import bacc
from concourse.bass_utils import run_bass_kernel_spmd
from concourse.masks import make_identity

F32 = mybir.dt.float32
BF16 = mybir.dt.bfloat16
AF = mybir.ActivationFunctionType
ALU = mybir.AluOpType
BF = ml_dtypes.bfloat16

DIM = 2048
HQ = 16            # query heads
HK = 4             # kv heads
HD = 128           # head dim
KV = HD * HK       # 512
N = 2048           # sequence length
FF = 4 * DIM       # 8192 mlp hidden
EPS = 1e-6
ROPE_BASE = 10000.0
SCALE = HD ** -0.5

NCORES = 8
RG = [list(range(NCORES))]
NCH = N // 128       # 16 sequence chunks
NIC = DIM // 128     # 16 feature chunks
FSH = FF // NCORES   # 1024 mlp hidden per core
FSC = FSH // 128     # 8 f-chunks per core
NEG = -1e30

KSZ = HK * 128 * 256          # kT_own elems (bf16)
VSZ = 2 * 128 * KV            # v elems

# MLP n-chunking for the ReduceScatter: per-core row-offsets/sizes within the
# core's 256 local rows. Small tail chunk so the final RS is cheap.
CH_O = [0, 64, 128, 192, 224]
CH_P = [64, 64, 64, 32, 32]
CH_B = [0, 512, 1024, 1536, 1792]   # 8*p cumulative: global col base
NG = len(CH_P)

# core c owns stripes (c, 15-c); local rows = [stripe_c | stripe_{15-c}]
# global s-chunk j lives on core own(j), slot slot(j):
def _owner(j):
    return (j, 0) if j < NCH // 2 else (NCH - 1 - j, 1)


def _build_kernel():
    nc = bacc.Bacc(None, target_bir_lowering=False)

    x_rows = nc.dram_tensor("x_rows", [2, 128, DIM], F32, kind="ExternalInput")
    xT_ext = nc.dram_tensor("xT", [NIC, 128, 256], BF16, kind="ExternalInput")
    rtab = nc.dram_tensor("rtab", [2, 2, 128, 256], F32, kind="ExternalInput")
    masks = nc.dram_tensor("masks", [6, 128, 512], BF16, kind="ExternalInput")
    wqkvT = nc.dram_tensor("wqkvT", [DIM, 3072], BF16, kind="ExternalInput")
    woT = nc.dram_tensor("woT", [DIM, DIM], BF16, kind="ExternalInput")
    w1S = nc.dram_tensor("w1S", [FSC, 128, DIM], BF16, kind="ExternalInput")
    w3S = nc.dram_tensor("w3S", [FSC, 128, DIM], BF16, kind="ExternalInput")
    w2T = nc.dram_tensor("w2T", [FSH, DIM], BF16, kind="ExternalInput")
    out_ext = nc.dram_tensor("out", [2, 128, DIM], F32, kind="ExternalOutput")

    with tile.TileContext(nc) as tc:
        _body(nc, tc, x_rows, xT_ext, rtab, masks,
              wqkvT, woT, w1S, w3S, w2T, out_ext)
    nc.compile()
    return nc


def _rope_psum(nc, rp, rtab_sb, pcur, sl, dst):
    """rope a [128, 512] psum tile (4 head-blocks) into dst [128, 512] bf16.
    rtab is pre-scaled by 1/rms so the output is the normed-roped value."""
    pv = pcur.rearrange("p (h t) -> p h t", t=128)
    cosT = rtab_sb[:, sl, 0, :].rearrange("p (h t) -> p h t", t=64)
    sinT = rtab_sb[:, sl, 1, :].rearrange("p (h t) -> p h t", t=64)
    t1 = rp.tile([128, 4, 64], F32, name="t1", tag="t1")
    t2 = rp.tile([128, 4, 64], F32, name="t2", tag="t2")
    t3 = rp.tile([128, 4, 64], F32, name="t3", tag="t3")
    t4 = rp.tile([128, 4, 64], F32, name="t4", tag="t4")
    nc.vector.tensor_mul(t1[:], pv[:, :, 0:64], cosT)
    nc.vector.tensor_mul(t2[:], pv[:, :, 64:128], sinT)
    nc.vector.tensor_mul(t3[:], pv[:, :, 0:64], sinT)
    nc.vector.tensor_mul(t4[:], pv[:, :, 64:128], cosT)
    dstv = dst.rearrange("p (h t) -> p h t", t=128)
    nc.vector.tensor_sub(dstv[:, :, 0:64], t1[:], t2[:])
    nc.vector.tensor_add(dstv[:, :, 64:128], t3[:], t4[:])


def _rmsnorm_to(nc, pool, out_bf, x_sb, slot, eps_tile, scratch):
    """out_bf[:, slot, :] = rmsnorm(x_sb[:, slot, :]) cast bf16.
    scratch: any writable [128, DIM] f32 AP whose contents may be clobbered."""
    ssq = pool.tile([128, 1], F32, name="ssq", tag="ssq")
    nc.scalar.activation(scratch, x_sb[:, slot, :], AF.Square, accum_out=ssq[:])
    rms = pool.tile([128, 1], F32, name="rms", tag="rms")
    nc.scalar.activation(rms[:], ssq[:], AF.Sqrt, bias=eps_tile[:], scale=1.0 / DIM)
    rinv = pool.tile([128, 1], F32, name="rinv", tag="rinv")
    nc.vector.reciprocal(rinv[:], rms[:])
    nc.vector.tensor_scalar_mul(out_bf[:, slot, :], x_sb[:, slot, :], rinv[:])


def _body(nc, tc, x_rows, xT_ext, rtab, masks,
          wqkvT, woT, w1S, w3S, w2T, out_ext):
    import contextlib
    ctx = contextlib.ExitStack()
    with ctx:
        const = ctx.enter_context(tc.tile_pool(name="const", bufs=1))
        persist = ctx.enter_context(tc.tile_pool(name="persist", bufs=1))
        dram = ctx.enter_context(tc.tile_pool(name="dram", bufs=1, space="DRAM"))
        small = ctx.enter_context(tc.tile_pool(name="small", bufs=4))

        # constants + warmup operands first on gpsimd so the PE warmup isn't
        # queued behind big DMA issues
        ident = const.tile([128, 128], BF16)
        make_identity(nc, ident)
        eps_tile = const.tile([128, 1], F32)
        nc.gpsimd.memset(eps_tile[:], EPS)
        wrm = const.tile([128, 512], BF16)
        nc.gpsimd.memset(wrm[:], 1.0)

        # DRAM comm buffers
        agkv_in = dram.tile([KSZ + VSZ], BF16)
        agkv_out = dram.tile([NCORES, KSZ + VSZ], BF16, addr_space="Shared")
        # x2n^T shipped p-major per slot: ship/gather DMAs then move 4KB
        # contiguous runs per partition on both sides (small-run gathers
        # measured 4-8x below line rate and starved the MLP entry)
        agx_in = dram.tile([2, 128, NIC * 128], BF16)
        agx_outs = [dram.tile([NCORES, 128 * NIC * 128], BF16,
                              addr_space="Shared", name=f"agxo{s_}")
                    for s_ in range(2)]
        rs_in = [dram.tile([8 * CH_P[g], DIM], BF16, name=f"rsin{g}")
                 for g in range(NG)]
        rs_out = [dram.tile([CH_P[g], DIM], BF16, name=f"rsout{g}")
                  for g in range(NG)]

        # persistent SBUF
        h_sb = persist.tile([128, 2, DIM], F32)       # post-attention residual
        x2nT = persist.tile([128, 2, NIC, 128], BF16)   # slot-major

        # attention-phase pool: closed before the MLP pool allocates
        att_ctx = contextlib.ExitStack()
        ph1 = att_ctx.enter_context(tc.tile_pool(name="ph1", bufs=1))
        qkv_ctx = contextlib.ExitStack()
        qkvp = qkv_ctx.enter_context(tc.tile_pool(name="qkvp", bufs=1))
        # Prologue DMA priority: the first K/V matmuls gate only on xT half 0
        # and wkv quarter 0 (~1.5MB), so those go first on their queues; x/rtab
        # (rope chain, needed ~t+20us) next; masks/bias_wo late. wkv/xT are
        # separate tiles per chunk so tile-granularity deps don't serialize.
        xT_sb = [qkvp.tile([128, 8, 256], BF16, name=f"xT{h}", tag=f"xT{h}")
                 for h in range(2)]
        wkv_ctx = contextlib.ExitStack()
        wkvres = wkv_ctx.enter_context(tc.tile_pool(name="wkvres", bufs=1))
        wkv_sb = [wkvres.tile([128, 4, 1024], BF16, name=f"wkv{h}", tag=f"wkv{h}")
                  for h in range(4)]
        rtab_sb = qkvp.tile([128, 2, 2, 256], F32)
        x_sb = ph1.tile([128, 2, DIM], F32)           # own rows [slotL | slotH]

        xT_v = xT_ext.rearrange("i p c -> p i c")
        wkv_v = wqkvT[:, 0:1024].rearrange("(i p) c -> p i c", p=128)
        nc.sync.dma_start(xT_sb[0][:], xT_v[:, 0:8, :])
        nc.scalar.dma_start(xT_sb[1][:], xT_v[:, 8:16, :])
        nc.gpsimd.dma_start(wkv_sb[0][:], wkv_v[:, 0:4, :])
        nc.sync.dma_start(wkv_sb[1][:], wkv_v[:, 4:8, :])
        nc.gpsimd.dma_start(wkv_sb[2][:], wkv_v[:, 8:12, :])
        nc.scalar.dma_start(wkv_sb[3][:], wkv_v[:, 12:16, :])
        nc.sync.dma_start(x_sb[:, 0, :], x_rows[0])
        nc.scalar.dma_start(x_sb[:, 1, :], x_rows[1])
        nc.gpsimd.dma_start(rtab_sb[:], rtab.rearrange("s c p t -> p s c t"))
        # PE warmup: DMA-independent matmuls raise HAM to K=8/8 while the
        # prologue DMAs run; result sunk to DRAM to stay live.
        warm_sink = dram.tile([128, 1], F32)
        with (
            tc.tile_pool(name="warmp", bufs=1, space="PSUM") as warmp,
            tc.tile_pool(name="warms", bufs=1) as warms,
        ):
            wps = warmp.tile([128, 512], F32)
            for wi in range(8):
                nc.tensor.matmul(wps[:], ident[:], wrm[:],
                                 start=True, stop=True)
            wsb = warms.tile([128, 1], F32)
            nc.vector.tensor_copy(wsb[:], wps[:, 0:1])
            nc.sync.dma_start(warm_sink[:], wsb[:])
        # per-slot 1/rms of x, folded into the rope tables (k, q) and the V
        # psum copy: rmsnorm(x) @ W == rinv * (x @ W) row-wise.
        rinv_s = []
        for s in range(2):
            ssq = small.tile([128, 1], F32, name="ssq", tag="ssq")
            nc.scalar.activation(h_sb[:, s, :], x_sb[:, s, :], AF.Square,
                                 accum_out=ssq[:])
            rms = small.tile([128, 1], F32, name="rms", tag="rms")
            nc.scalar.activation(rms[:], ssq[:], AF.Sqrt, bias=eps_tile[:],
                                 scale=1.0 / DIM)
            rinv = qkvp.tile([128, 1], F32, name=f"rinv{s}", tag=f"rinv{s}")
            nc.vector.reciprocal(rinv[:], rms[:])
            nc.vector.tensor_scalar_mul(rtab_sb[:, s, :, :],
                                        rtab_sb[:, s, :, :], rinv[:])
            rinv_s.append(rinv)

        # ===== phase 2a: K/V projections + rope-k + early fused AllGather ====
        # qkv_rows[:, slot, 0:2048]=roped q, [2048:2560]=roped k, [2560:3072]=v
        # wqkvT col order: [k(512) | v(512) | q(2048)]
        qkv_rows = qkvp.tile([128, 2, 3072], BF16)
        kT_own = qkvp.tile([128, HK, 256], BF16)
        q_roped = ph1.tile([128, HQ, 256], BF16)

        with (
            tc.tile_pool(name="pkv", bufs=1, space="PSUM") as pkv,
            tc.tile_pool(name="rp", bufs=2) as rp,
        ):
            ps = [pkv.tile([128, 512], F32, name=f"pkv{u}", tag=f"pkv{u}")
                  for u in range(4)]          # (oc, slot): oc0=k, oc1=v
            for ic in range(NIC):
                for sl in range(2):
                    for oi in range(2):
                        nc.tensor.matmul(
                            ps[oi * 2 + sl][:],
                            xT_sb[ic // 8][:, ic % 8, sl * 128:(sl + 1) * 128],
                            wkv_sb[ic // 4][:, ic % 4, oi * 512:(oi + 1) * 512],
                            start=(ic == 0), stop=(ic == NIC - 1))
            # k: rope then transpose + ship (K first: attention scores gate on
            # it); v: rinv-scaled copy, ship second; ONE fused AllGather. The
            # ships + trigger ride the gpsimd queue, which is idle here, so the
            # wq weight stream on sync/scalar is never blocked behind them.
            for sl in range(2):
                _rope_psum(nc, rp, rtab_sb, ps[0 + sl],
                           sl, qkv_rows[:, sl, 2048:2560])
            with tc.tile_pool(name="tpk", bufs=2, space="PSUM") as tpk:
                for sl in range(2):
                    for kh in range(HK):
                        tp = tpk.tile([128, 128], BF16, name="tp_k",
                                      tag="tp_k")
                        nc.tensor.transpose(
                            tp[:],
                            qkv_rows[:, sl,
                                     2048 + kh * 128:2048 + (kh + 1) * 128],
                            ident[:])
                        nc.vector.tensor_copy(
                            kT_own[:, kh, sl * 128:(sl + 1) * 128], tp[:])
            nc.gpsimd.dma_start(
                agkv_in[0:KSZ].rearrange("(k d n) -> d k n", k=HK, d=128),
                kT_own[:])
            for sl in range(2):
                nc.vector.tensor_scalar_mul(qkv_rows[:, sl, 2560:3072],
                                            ps[2 + sl][:], rinv_s[sl][:])
            nc.gpsimd.dma_start(
                agkv_in[KSZ:KSZ + VSZ].rearrange("(t2 t k) -> t t2 k",
                                                 t2=2, t=128),
                qkv_rows[:, :, 2560:3072])
            nc.gpsimd.collective_compute(
                "AllGather", ALU.bypass, replica_groups=RG,
                ins=[agkv_in.opt()], outs=[agkv_out.opt()])
        mask_sb = ph1.tile([128, 6, 512], BF16)
        nc.gpsimd.dma_start(mask_sb[:], masks.rearrange("k p q -> p k q"))

        wkv_ctx.close()
        # ===== phase 2b: Q projections + rope + transposes (overlap AG) ====
        with (
            tc.tile_pool(name="wq", bufs=5) as wqp,
            tc.tile_pool(name="pq", bufs=1, space="PSUM") as pq,
            tc.tile_pool(name="rp2", bufs=2) as rp2,
        ):
            psq = [pq.tile([128, 512], F32, name=f"pq{u}", tag=f"pq{u}")
                   for u in range(8)]         # (oc, slot)
            for ic in range(NIC):
                w_t = wqp.tile([128, 2048], BF16, name="wq_t", tag="wqt")
                eng = nc.sync if ic % 2 == 0 else nc.scalar
                eng.dma_start(
                    w_t[:], wqkvT[ic * 128:(ic + 1) * 128, 1024:3072])
                for oi in range(4):
                    for sl in range(2):
                        nc.tensor.matmul(
                            psq[oi * 2 + sl][:],
                            xT_sb[ic // 8][:, ic % 8, sl * 128:(sl + 1) * 128],
                            w_t[:, oi * 512:(oi + 1) * 512],
                            start=(ic == 0), stop=(ic == NIC - 1))
            for oi in range(4):
                for sl in range(2):
                    _rope_psum(nc, rp2, rtab_sb, psq[oi * 2 + sl],
                               sl, qkv_rows[:, sl, oi * 512:(oi + 1) * 512])
        # transposes: q -> q_roped [d, h, n] (head-ascending: scores gate on
        # low heads first)
        with tc.tile_pool(name="tpq", bufs=3, space="PSUM") as tpq:
            for h in range(HQ):
                for sl in range(2):
                    tp = tpq.tile([128, 128], BF16, name="tp_q", tag="tp_q")
                    nc.tensor.matmul(tp[:], qkv_rows[:, sl, h * 128:(h + 1) * 128],
                                     ident[:], is_transpose=True)
                    nc.vector.tensor_copy(q_roped[:, h, sl * 128:(sl + 1) * 128], tp[:])
        qkv_ctx.close()
        # woT resident for the slot-major wo phase; DMAs run during attention
        wores = att_ctx.enter_context(tc.tile_pool(name="wores", bufs=1))
        woT_sb = wores.tile([128, NIC, DIM], BF16)
        for hf in range(4):
            eng = nc.sync if hf % 2 == 0 else nc.gpsimd
            eng.dma_start(
                woT_sb[:, hf * 4:(hf + 1) * 4, :],
                woT.rearrange("(i p) o -> p i o", p=128)[:, hf * 4:(hf + 1) * 4, :])

        # ============ phase 4: gather K/V into SBUF (rank-major layouts) ====
        # kT_full[:, kh, r, slot*128+t] = rank r's K slot cols; unit code
        # indexes via _owner(j) -> (r, slot). K gathers first (scores gate on
        # them), spread over 3 queues.
        kT_full = ph1.tile([128, HK, NCORES, 256], BF16)
        v_aug = ph1.tile([128, NCORES, 2, HK, 132], BF16)
        # K/V gathers ride gpsimd (idle post-trigger); sync/scalar keep
        # streaming the wq/woT weights.
        nc.gpsimd.memset(v_aug[:, :, :, :, 128:129], 1.0)
        kengs = [nc.gpsimd, nc.sync]
        for r in range(NCORES):
            kengs[r % 2].dma_start(
                kT_full[:, :, r, :],
                agkv_out[r][0:KSZ].rearrange("(k d n) -> d k n", k=HK, d=128))
        vengs = [nc.gpsimd, nc.sync, nc.scalar]
        for r in range(NCORES):
            vsrc = agkv_out[r][KSZ:KSZ + VSZ].rearrange(
                "(t2 t k d) -> t t2 k d", t2=2, t=128, k=HK)
            for sl2 in range(2):
                vengs[(2 * r + sl2) % 3].dma_start(v_aug[:, r, sl2, :, 0:128],
                                                   vsrc[:, sl2, :, :])

        # ============ phase 5+6: slot-split attention + wo + AG ============
        # All slot-L work runs first (L scores/AV for every head -> wo-L ->
        # norm -> ship -> AG-L), then the slot-H pass covers the AG-L flight
        # entirely, so the MLP's L-half is unblocked the moment wo-H finishes.
        # Scores are hoisted AV_DEPTH heads ahead of AV so the scalar/vector
        # exp+mask chain pipelines ahead of the AV matmuls.
        attn = ph1.tile([128, 2, DIM], BF16)     # row-major attn out (normalized)
        attnT = ph1.tile([128, NIC, 256], BF16)
        x2n = ph1.tile([128, 2, DIM], BF16)
        AV_DEPTH = 3
        with (
            tc.tile_pool(name="ps_sc", bufs=2, space="PSUM") as ps_sc,
            tc.tile_pool(name="ps_av", bufs=1, space="PSUM") as ps_av,
            tc.tile_pool(name="tp_at", bufs=1, space="PSUM") as tp_at,
            tc.tile_pool(name="att_sb", bufs=4 * (AV_DEPTH + 1)) as att_sbp,
            tc.tile_pool(name="att_tmp", bufs=3) as att_tmp,
            tc.tile_pool(name="po", bufs=1, space="PSUM") as po,
        ):
            # single-bank manual rings (PSUM pool tiles are bank-quantized)
            av_ring = ps_av.tile([128, 2, 132], F32)
            tp_ring = tp_at.tile([128, 2, 128], BF16)
            def scores_phase(h, s):
                kh = h % HK
                nquad = 2 if s == 0 else 4
                att_tiles = []
                for qa in range(nquad):
                    sc = ps_sc.tile([128, 512], F32, name="sc", tag="sc")
                    for u in range(4):
                        j = qa * 4 + u
                        rk, sk = _owner(j)
                        nc.tensor.matmul(
                            sc[:, u * 128:(u + 1) * 128],
                            kT_full[:, kh, rk, sk * 128:(sk + 1) * 128],
                            q_roped[:, h, s * 128:(s + 1) * 128],
                            start=True, stop=True)
                    tmp = att_tmp.tile([128, 512], BF16, name="mtmp", tag="mtmp")
                    nc.scalar.activation(tmp[:], sc[:], AF.Exp, scale=SCALE)
                    att = att_sbp.tile([128, 512], BF16, name="attP", tag="attP")
                    nc.vector.tensor_mul(att[:], tmp[:],
                                         mask_sb[:, 2 * s + qa, :])
                    att_tiles.append(att)
                return att_tiles

            def av_phase(h, s, att_tiles):
                kh = h % HK
                last = 7 if s == 0 else NCH - 1
                av = av_ring[:, h % 2, :]
                for qa, att in enumerate(att_tiles):
                    for u in range(4):
                        j = qa * 4 + u
                        rk, sk = _owner(j)
                        nc.tensor.matmul(
                            av[:, 0:129], att[:, u * 128:(u + 1) * 128],
                            v_aug[:, rk, sk, kh, 0:129],
                            start=(j == 0), stop=(j == last))
                # normalize by denominator (col 128), then transpose this
                # head's column into attnT right away
                rd = small.tile([128, 1], F32, name="rd", tag="rd")
                nc.vector.reciprocal(rd[:], av[:, 128:129])
                nc.vector.tensor_scalar_mul(
                    attn[:, s, h * 128:(h + 1) * 128], av[:, 0:128], rd[:])
                tp = tp_ring[:, h % 2, :]
                nc.tensor.transpose(
                    tp, attn[:, s, h * 128:(h + 1) * 128], ident[:])
                nc.vector.tensor_copy(
                    attnT[:, h, s * 128:(s + 1) * 128], tp)

            def wo_slot(s):
                pso = [po.tile([128, 512], F32, name=f"pso{i}", tag=f"pso{i}")
                       for i in range(4)]
                for ic in range(NIC):
                    for oc in range(4):
                        nc.tensor.matmul(
                            pso[oc][:],
                            attnT[:, ic, s * 128:(s + 1) * 128],
                            woT_sb[:, ic, oc * 512:(oc + 1) * 512],
                            start=(ic == 0), stop=(ic == NIC - 1))
                for oc in range(4):
                    nc.vector.tensor_add(
                        h_sb[:, s, oc * 512:(oc + 1) * 512],
                        pso[oc][:], x_sb[:, s, oc * 512:(oc + 1) * 512])
                _rmsnorm_to(nc, small, x2n, h_sb, s, eps_tile, x_sb[:, s, :])
                for ic in range(NIC):
                    tp = tp_ring[:, ic % 2, :]
                    nc.tensor.transpose(
                        tp, x2n[:, s, ic * 128:(ic + 1) * 128], ident[:])
                    nc.vector.tensor_copy(x2nT[:, s, ic, :], tp)
                eng = nc.sync if s == 0 else nc.scalar
                eng.dma_start(
                    agx_in[s].rearrange("p (i t) -> p i t", i=NIC),
                    x2nT[:, s])
                nc.gpsimd.collective_compute(
                    "AllGather", ALU.bypass, replica_groups=RG,
                    ins=[agx_in[s].opt()], outs=[agx_outs[s].opt()])

            for s in range(2):
                pend = []
                for h in range(HQ):
                    pend.append((h, scores_phase(h, s)))
                    if len(pend) > AV_DEPTH:
                        hh, tiles = pend.pop(0)
                        av_phase(hh, s, tiles)
                for hh, tiles in pend:
                    av_phase(hh, s, tiles)
                wo_slot(s)

        # ============ phase 8: MLP (TP, FF/8) with chunked RS ============
        # x2nT_full is laid out g-chunk-major: cols [CH_B[g] + r*CH_P[g] + q]
        # so the h2 matmul rhs is contiguous. Gather DMAs run g-ascending
        # (g0/g1 need only the slot-L AllGather) across all 5 queues; the w2
        # weights load AFTER the gathers, on the tensor/vector queues, so they
        # don't delay the first h2 matmuls.
        att_ctx.close()
        mlpw = ctx.enter_context(tc.tile_pool(name="mlpw", bufs=1, side="right"))
        mlp = ctx.enter_context(tc.tile_pool(name="mlp", bufs=1, side="right"))
        # rank-major gathered activations: [p, rank, slot, ic, t] — the
        # per-(rank,slot) gather DMA moves one 512KB contiguous-per-partition
        # block at full line rate.
        x2nT_full = mlp.tile([128, NCORES, 2, NIC, 128], BF16)
        gengs = [nc.sync, nc.gpsimd, nc.scalar]

        def _gather_slot(s):
            # slot L rides sync alone: gpsimd/scalar are blocked behind the
            # slot-H ship + AG trigger until wo-H data is ready, but sync is
            # free right after ship-L, so these run the moment AG-L lands.
            engs = [nc.sync] if s == 0 else [nc.sync, nc.gpsimd, nc.scalar]
            for r in range(NCORES):
                engs[r % len(engs)].dma_start(
                    x2nT_full[:, r, s],
                    agx_outs[s][r].rearrange("(p i t) -> p i t", p=128, i=NIC))

        # w1/w3 stream with a 4-deep software-pipelined prefetch across block
        # boundaries; the first tiles are issued before the gathers so the
        # first h2 matmuls aren't queued behind them.
        w13p = mlpw
        W13_STEPS = [(b, f) for b in range(4) for f in range(FSC)]

        def _w13_issue(k):
            b, f = W13_STEPS[k]
            w1_t = w13p.tile([128, NIC, 128], BF16, name="w1_t", tag="w1", bufs=3)
            nc.sync.dma_start(w1_t.rearrange("p i f -> p (i f)"), w1S[f])
            w3_t = w13p.tile([128, NIC, 128], BF16, name="w3_t", tag="w3", bufs=3)
            nc.gpsimd.dma_start(w3_t.rearrange("p i f -> p (i f)"), w3S[f])
            return (w1_t, w3_t)

        from collections import deque
        w13_q = deque(_w13_issue(k) for k in range(3))

        # slot-L gathers gate only on the slot-L AllGather; slot-H is gathered
        # after block 0's h2 pass so its semaphore wait never blocks work
        # queued behind it. The w13 tiles above were issued first so they
        # prefetch during the attention tail.
        _gather_slot(0)
        w2_sb = mlpw.tile([128, FSC, DIM], BF16)
        for hf in range(2):
            eng = nc.scalar if hf == 0 else nc.gpsimd
            eng.dma_start(
                w2_sb[:, hf * 4:(hf + 1) * 4, :],
                w2T.rearrange("(f p) o -> p f o", p=128)[:, hf * 4:(hf + 1) * 4, :])

        # h2 runs in 4 column-blocks of 512 = (slot, 64-col half) x 8 ranks;
        # block 3 covers RS chunks 3+4 (strided w2 lhsT) for a small RS tail.
        BLK_CH = [[0], [1], [2], [3, 4]]
        with (
            tc.tile_pool(name="ps_y", bufs=2, space="PSUM") as ps_y,
            tc.tile_pool(name="h2p", bufs=2) as h2p,
            tc.tile_pool(name="ps_w2", bufs=2, space="PSUM") as ps_w2,
            tc.tile_pool(name="rs_sb", bufs=2) as rs_sbp,
            tc.tile_pool(name="rp3p", bufs=1) as rp3p,
        ):
            for b in range(4):
                sb_, th = b // 2, b % 2
                h2T = h2p.tile([128, FSC, 512], BF16, name="h2T", tag="h2T")
                for f in range(FSC):
                    w1_t, w3_t = w13_q.popleft()
                    k = b * FSC + f
                    if k + 3 < len(W13_STEPS):
                        w13_q.append(_w13_issue(k + 3))
                    y1 = ps_y.tile([128, 512], F32, name="y1", tag="y1")
                    y3 = ps_y.tile([128, 512], F32, name="y3", tag="y3")
                    for ic in range(NIC):
                        rhs = x2nT_full[:, :, sb_, ic, th * 64:(th + 1) * 64]
                        nc.tensor.matmul(y1[:], w1_t[:, ic, :], rhs,
                                         start=(ic == 0), stop=(ic == NIC - 1))
                        nc.tensor.matmul(y3[:], w3_t[:, ic, :], rhs,
                                         start=(ic == 0), stop=(ic == NIC - 1))
                    sg = rs_sbp.tile([128, 512], BF16, name="sg", tag="sg")
                    nc.scalar.activation(sg[:], y1[:], AF.Sigmoid)
                    sil = rs_sbp.tile([128, 512], F32, name="sil", tag="sil")
                    nc.vector.scalar_tensor_tensor(
                        sil[:], y1[:], 1.0, sg[:], op0=ALU.mult, op1=ALU.mult)
                    nc.vector.tensor_mul(h2T[:, f, :], sil[:], y3[:])
                if b == 0:
                    _gather_slot(1)
                # w2 + RS per chunk of this block
                h2T_r = h2T.rearrange("p f (r t) -> p f r t", t=64)
                for g in BLK_CH[b]:
                    nrow = 8 * CH_P[g]
                    if CH_P[g] != 64:
                        # stationary matmul operand needs one contiguous free
                        # dim: repack this chunk's (rank, 32) columns
                        off = CH_O[g] % 64
                        rp3 = rp3p.tile([128, FSC, 256], BF16,
                                        name="rp3", tag="rp3")
                        for f in range(FSC):
                            nc.vector.tensor_copy(
                                rp3[:, f, :].rearrange("p (r t) -> p r t", t=32),
                                h2T_r[:, f, :, off:off + 32])
                    for qi in range(nrow // 128):
                        if CH_P[g] == 64:
                            lhs_f = lambda f, qi=qi: h2T[:, f, qi * 128:(qi + 1) * 128]
                        else:
                            lhs_f = lambda f, qi=qi: rp3[:, f, qi * 128:(qi + 1) * 128]
                        for oc in range(4):
                            pw = ps_w2.tile([128, 512], F32, name="pw", tag="pw")
                            for f in range(FSC):
                                nc.tensor.matmul(
                                    pw[:], lhs_f(f),
                                    w2_sb[:, f, oc * 512:(oc + 1) * 512],
                                    start=(f == 0), stop=(f == FSC - 1))
                            ob = rs_sbp.tile([128, 512], BF16, name="ob", tag="ob")
                            if (qi * 4 + oc) % 2 == 0:
                                nc.vector.tensor_copy(ob[:], pw[:])
                            else:
                                nc.scalar.copy(ob[:], pw[:])
                            nc.sync.dma_start(
                                rs_in[g][qi * 128:(qi + 1) * 128,
                                         oc * 512:(oc + 1) * 512], ob[:])
                    nc.gpsimd.collective_compute(
                        "ReduceScatter", ALU.add, replica_groups=RG,
                        ins=[rs_in[g].opt()], outs=[rs_out[g].opt()])

        # ============ phase 9: final residual + output (per RS chunk) =======
        rs_res = mlp.tile([128, 2, DIM], BF16)
        out_sb = mlp.tile([128, 2, DIM], F32)
        for g in range(NG):
            s, lo, p = CH_O[g] // 128, CH_O[g] % 128, CH_P[g]
            pr = slice(lo, lo + p)
            eng = nc.sync if g % 2 == 0 else nc.gpsimd
            eng.dma_start(rs_res[pr, s, :], rs_out[g][:])
            nc.vector.tensor_add(out_sb[pr, s, :], rs_res[pr, s, :], h_sb[pr, s, :])
            eng.dma_start(out_ext[s, pr, :], out_sb[pr, s, :])


# ============================ host side ============================

def _perm(nheads):
    p = []
    for h in range(nheads):
        base = h * HD
        p.extend(range(base, base + HD, 2))
        p.extend(range(base + 1, base + HD, 2))
    return np.array(p)


def _rope_tabs(pos):
    inv = 1.0 / (ROPE_BASE ** (np.arange(0, HD, 2, dtype=np.float32) / HD))
    f = np.outer(pos.astype(np.float32), inv)        # [n, 64]
    return np.cos(f).T.astype(np.float32), np.sin(f).T.astype(np.float32)


def _mask_for(stripe, j):
    """multiplicative mask [128 s, 128 q] for s-chunk j vs q-stripe `stripe`"""
    if j < stripe:
        return np.ones((128, 128), np.float32)
    if j > stripe:
        return np.zeros((128, 128), np.float32)
    i = np.arange(128)
    return np.where(i[:, None] <= i[None, :], 1.0, 0.0).astype(np.float32)


def _wimg(wshard):
    """[1024, 2048] w-shard -> SBUF images [8 f-chunks, 128 part(i%128), 16*128]
    img[f][p, ic*128+t] = w.T[ic*128+p, f*128+t]"""
    wT = wshard.T                      # [2048 i, 1024 f]
    img = wT.reshape(NIC, 128, FSC, 128).transpose(2, 1, 0, 3).reshape(FSC, 128, DIM)
    return np.ascontiguousarray(img).astype(BF)


_CACHED_NC = None


def _get_nc():
    global _CACHED_NC
    if _CACHED_NC is None:
        _CACHED_NC = _build_kernel()
    return _CACHED_NC


def _prep_in_maps(inputs):
    f32 = lambda a: np.ascontiguousarray(np.asarray(a), dtype=np.float32)
    x = f32(inputs["x"])[0]                  # [N, DIM]
    g_attn, g_mlp = f32(inputs["g_attn"]), f32(inputs["g_mlp"])
    pq, pk = _perm(HQ), _perm(HK)
    wq = f32(inputs["wq"])[pq] * g_attn[None, :]
    wk = f32(inputs["wk"])[pk] * g_attn[None, :]
    wv = f32(inputs["wv"]) * g_attn[None, :]
    wo = f32(inputs["wo"])
    w1 = f32(inputs["w1"]) * g_mlp[None, :]
    w3 = f32(inputs["w3"]) * g_mlp[None, :]
    w2 = f32(inputs["w2"])
    wqkv = np.concatenate([wk, wv, wq], 0)         # [3072, 2048] (k|v|q)
    xT = np.ascontiguousarray(x.T)                 # [DIM, N]
    shared = {
        "wqkvT": np.ascontiguousarray(wqkv.T).astype(BF),
        "woT": np.ascontiguousarray(wo.T).astype(BF),
    }
    in_maps = []
    for c in range(NCORES):
        sl, sh = c, NCH - 1 - c
        pos = np.concatenate([np.arange(sl * 128, (sl + 1) * 128),
                              np.arange(sh * 128, (sh + 1) * 128)])
        cos, sin = _rope_tabs(pos)           # [64, 256] feature-major
        # row-major per-slot tables tiled 4x along free: [2 slot, 2 (cos,sin), 128, 256]
        rt = np.zeros((2, 2, 128, 256), np.float32)
        for slot_i in range(2):
            cr = cos[:, slot_i * 128:(slot_i + 1) * 128].T    # [128, 64]
            sr = sin[:, slot_i * 128:(slot_i + 1) * 128].T
            rt[slot_i, 0] = np.tile(cr, (1, 4))
            rt[slot_i, 1] = np.tile(sr, (1, 4))
        # masks [6, 128, 512]: 2 L-quad tiles (chunks 0..7 vs stripe sl) then
        # 4 H-quad tiles (chunks 0..15 vs stripe sh)
        m = np.zeros((6, 128, 512), np.float32)
        for qa in range(2):
            for u_ in range(4):
                m[qa, :, u_ * 128:(u_ + 1) * 128] = _mask_for(sl, qa * 4 + u_)
        for qb in range(4):
            for u_ in range(4):
                m[2 + qb, :, u_ * 128:(u_ + 1) * 128] = _mask_for(sh, qb * 4 + u_)
        # x^T for this core's 256 rows, feature-chunked: [NIC, 128, 256]
        xTc = xT[:, pos].reshape(NIC, 128, 256)
        im = {
            "x_rows": np.stack([x[sl * 128:(sl + 1) * 128],
                                x[sh * 128:(sh + 1) * 128]]),
            "xT": np.ascontiguousarray(xTc).astype(BF),
            "rtab": rt,
            "masks": m.astype(BF),
            "wqkvT": shared["wqkvT"], "woT": shared["woT"],
            "w1S": _wimg(w1[c * FSH:(c + 1) * FSH]),
            "w3S": _wimg(w3[c * FSH:(c + 1) * FSH]),
            "w2T": np.ascontiguousarray(
                w2[:, c * FSH:(c + 1) * FSH].T).astype(BF),
        }
        in_maps.append(im)
    return in_maps


def kernel(**inputs) -> np.ndarray:
    nc = _get_nc()
    in_maps = _prep_in_maps(inputs)
    res = run_bass_kernel_spmd(nc, in_maps, core_ids=list(range(NCORES)))
    out = np.empty((1, N, DIM), np.float32)
    for c in range(NCORES):
        o = res.results[c]["out"]            # [2, 128, DIM]
        out[0, c * 128:(c + 1) * 128] = o[0]
        out[0, (NCH - 1 - c) * 128:(NCH - c) * 128] = o[1]
    return out
